# revision 1
# baseline (speedup 1.0000x reference)
"""Multi-head causal attention (b=4, n=2048, d=1024, h=16) on 8 trn2 cores.

Sharding: data-parallel over batch (4) x tensor-parallel over heads (2 groups
of 8 heads).  Core c handles batch c//2, heads 8*(c%2)..8*(c%2)+8.

Per-core dataflow (all matmuls in float32r: full PE rate, ~2e-4 rel err):
  xs[n] [128,8,512]   = x[b].T strip           (streamed per 512-col strip)
  w{q,k,v}T [1024,512] = W.T[:, group]         (host-prepared)
  KT [512,2048]       = wkT.T @ xT             (K transposed: [head*dim, n])
  Vp [2048, 8*65]     = xT.T @ wvT (+ ones col)
  QTs [512,512]       = wqT.T @ xs[n]          (per strip)
  per (head-pair, strip):
    S^T blocks [128 nk, 512 nq] = KT_h_blk.T @ QTs_h  (K=64 contraction;
      the pair's QK matmuls alternate PE row groups 0-63/64-127 so they
      stream concurrently; 2 blocks per PSUM chunk -> one exp(S/8) each)
    causal: 0/1 triangular multiply on the diagonal 128x128 sub-block (DVE,
      SBUF); QK and PV skip fully-masked column ranges
    [O^T; sums] accumulated in PSUM = [V|1]_blk.T @ P^T_blk over k blocks
    normalize on device: O^T * bcast(1/sums); the partition-broadcast runs
    on GPSIMD (raw sbuf tensors), then -> DRAM (Pool SWDGE)
  host gather: out[b, :, group] = outT.T

Each strip's projection groups (KT/Vp/QTs of that strip) are woven into its
own attention emission - one group per exp chunk - so the PE stays fed while
ACT (the attention bottleneck) drains the exp queue; per pair t only
[KT m=t, QT m=t] must precede it, and the new Vp blocks are woven at
2/chunk during pair 0 ahead of the diagonal PV matmuls that read them.
"""

import numpy as np

import concourse.bacc as bacc
import concourse.mybir as mybir
import concourse.tile as tile
from concourse import bass_utils
from concourse.bass_interp import get_hw_module

N_CORES = 8
B, N, D = 4, 2048, 1024
HEADS = 16
HPC = 8            # heads per core
HD = 64            # head dim
GD = HPC * HD      # 512 weight columns per core
KC = D // 128      # 8 contraction chunks of in_dim
NB = N // 128      # 16 key blocks
NSTRIP = N // 512  # 4 query strips
CH = 2             # S^T key-blocks per PSUM chunk / exp call
PSS_BUFS = 2       # PSUM: 2*2 banks S chunks + 1 proj + 2 PV accum = 7 of 8

f32 = mybir.dt.float32
f32r = mybir.dt.float32r
EXP = mybir.ActivationFunctionType.Exp


def build_program():
    nc = bacc.Bacc("TRN2", target_bir_lowering=False, debug=False,
                   num_devices=N_CORES)
    xT = nc.dram_tensor("xT", [D, N], f32r, kind="ExternalInput").ap()
    wqT = nc.dram_tensor("wqT", [D, GD], f32r, kind="ExternalInput").ap()
    wkT = nc.dram_tensor("wkT", [D, GD], f32r, kind="ExternalInput").ap()
    wvT = nc.dram_tensor("wvT", [D, GD], f32r, kind="ExternalInput").ap()
    tri01 = nc.dram_tensor("tri01", [128, 128], f32, kind="ExternalInput").ap()
    outT = nc.dram_tensor("outT", [GD, N], f32, kind="ExternalOutput").ap()
    # raw (non-pool) sbuf tensors: partition_broadcast needs concrete APs
    rec_raw = [nc.alloc_sbuf_tensor(f"rec_raw{i}", [1, 512], f32).ap()
               for i in range(2)]
    rb_raw = [nc.alloc_sbuf_tensor(f"rb_raw{i}", [64, 512], f32).ap()
              for i in range(2)]

    with tile.TileContext(nc) as tc:
        with (
            tc.tile_pool(name="xs", bufs=2) as xs_pool,
            tc.tile_pool(name="w", bufs=1) as w_pool,
            tc.tile_pool(name="big", bufs=1) as big_pool,
            tc.tile_pool(name="qt", bufs=3) as qt_pool,
            tc.tile_pool(name="pt", bufs=4) as pt_pool,
            tc.tile_pool(name="ot", bufs=3) as ot_pool,
            tc.tile_pool(name="small", bufs=1) as small_pool,
            tc.tile_pool(name="ps_s", bufs=PSS_BUFS, space="PSUM") as ps_s,
            tc.tile_pool(name="ps_proj", bufs=1, space="PSUM") as ps_proj,
            tc.tile_pool(name="ps_o", bufs=3, space="PSUM") as ps_o,
        ):
            wq_t = w_pool.tile([128, KC, GD], f32r, tag="wq")
            wk_t = w_pool.tile([128, KC, GD], f32r, tag="wk")
            wv_t = w_pool.tile([128, KC, GD], f32r, tag="wv")
            # interleave wk and strip-0 x chunks: the first KT projection
            # group consumes them in k order, so it starts after ~2 DMAs
            # instead of waiting behind all the weight loads
            xs0 = xs_pool.tile([128, KC, 512], f32r, tag="xs", name="xs0")
            for k in range(KC):
                nc.sync.dma_start(wk_t[:, k, :],
                                  wkT[k * 128:(k + 1) * 128, :])
                nc.sync.dma_start(xs0[:, k, :],
                                  xT[k * 128:(k + 1) * 128, 0:512])
            for wt, wd in ((wq_t, wqT), (wv_t, wvT)):
                for k in range(KC):
                    nc.sync.dma_start(wt[:, k, :],
                                      wd[k * 128:(k + 1) * 128, :])
            tri = small_pool.tile([128, 128], f32, tag="tri")
            nc.sync.dma_start(tri[:], tri01[:])
            # warm the ACT exp table while input DMAs stream
            warmup = small_pool.tile([1, 1], f32, tag="warmup")
            nc.vector.memset(warmup[:], 0.0)
            nc.scalar.activation(warmup[:], warmup[:], EXP)

            kt = big_pool.tile([128, 4, N], f32r, tag="kt")
            vp = big_pool.tile([128, NB, HPC, HD + 1], f32r, tag="vp")
            # ones column: init whole tile (contiguous memset); V copies
            # overwrite the value columns
            nc.vector.memset(
                vp[:].rearrange("p a b c -> p (a b c)").bitcast(f32), 1.0)

            def load_strip(n):
                xs = xs_pool.tile([128, KC, 512], f32r, tag="xs")
                for k in range(KC):
                    nc.sync.dma_start(
                        xs[:, k, :],
                        xT[k * 128:(k + 1) * 128, n * 512:(n + 1) * 512])
                return xs

            def proj_group(lhs_fn, rhs_fn, copy_out_fn):
                ps = ps_proj.tile([128, 512], f32, tag="psp", name="psp")
                for k in range(KC):
                    nc.tensor.matmul(ps[:], lhs_fn(k), rhs_fn(k),
                                     start=(k == 0), stop=(k == KC - 1))
                copy_out_fn(ps)

            def emit_strip_projections(n, xs, qts):
                """List of closures, one PE-sized projection group each."""
                groups = []
                for m in range(4):      # K^T rows m*128.. for strip n
                    groups.append(lambda m=m: proj_group(
                        lambda k, m=m: wk_t[:, k, m * 128:(m + 1) * 128],
                        lambda k: xs[:, k, :],
                        lambda ps, m=m: nc.vector.tensor_copy(
                            kt[:, m, n * 512:(n + 1) * 512], ps[:]),
                    ))
                for i in range(4):      # V blocks 4n+i
                    mt = 4 * n + i
                    groups.append(lambda mt=mt, i=i: proj_group(
                        lambda k, i=i: xs[:, k, i * 128:(i + 1) * 128],
                        lambda k: wv_t[:, k, :],
                        lambda ps, mt=mt: nc.vector.tensor_copy(
                            vp[:, mt, :, 0:HD],
                            ps[:].rearrange("p (h d) -> p h d", h=HPC)),
                    ))
                for m in range(4):      # Q^T strip n rows m*128..
                    groups.append(lambda m=m: proj_group(
                        lambda k, m=m: wq_t[:, k, m * 128:(m + 1) * 128],
                        lambda k: xs[:, k, :],
                        lambda ps, m=m: nc.vector.tensor_copy(
                            qts[:, m, :], ps[:]),
                    ))
                return groups

            def emit_pv(po, ptc, jj, h, qs, nblocks):
                for idx, j in enumerate(jj):
                    r = j - 4 * qs
                    nstart = 128 * r if r > 0 else 0
                    nc.tensor.matmul(
                        po[:, nstart:512],
                        vp[:, j, h, :],
                        ptc[:, idx, nstart:512],
                        start=(j == 0), stop=(j == nblocks - 1),
                    )

            def attention_pair(h0, qs, qts, weave_fn=None):
                """Heads (h0, h0+1): h0 on PE rows 0-63, h0+1 on rows 64-127.
                QK matmuls interleave the two heads so adjacent MMs hit
                disjoint row groups and stream concurrently."""
                nblocks = 4 * qs + 4
                heads = (h0, h0 + 1)
                m = h0 // 2
                po = {h: ps_o.tile([HD + 1, 512], f32, tag="po",
                                   name=f"po_h{h}")
                      for h in heads}
                pending = {h: None for h in heads}
                for c0 in range(0, nblocks, CH):
                    jj = list(range(c0, min(c0 + CH, nblocks)))
                    w = len(jj)
                    pss = {h: ps_s.tile([128, CH, 512], f32, tag="pss",
                                        name=f"pss_h{h}")
                           for h in heads}
                    for idx, j in enumerate(jj):
                        r = j - 4 * qs
                        nstart = 128 * r if 0 < r < 3 else 0
                        for h in heads:
                            p0 = (h % 2) * 64
                            nc.tensor.matmul(
                                pss[h][:, idx, nstart:512],
                                kt[p0:p0 + 64, m, j * 128:(j + 1) * 128],
                                qts[p0:p0 + 64, m, nstart:512],
                                start=True, stop=True,
                            )
                    ptc = {}
                    for h in heads:
                        ptc[h] = pt_pool.tile([128, CH, 512], f32r,
                                              tag="ptc", name=f"ptc_h{h}")
                        nc.scalar.activation(ptc[h][:, 0:w, :],
                                             pss[h][:, 0:w, :],
                                             EXP, scale=0.125)
                        # causal 0/1 mask on diagonal sub-blocks (SBUF)
                        for idx, j in enumerate(jj):
                            r = j - 4 * qs
                            if r >= 0:
                                nc.vector.tensor_mul(
                                    ptc[h][:, idx, r * 128:(r + 1) * 128],
                                    ptc[h][:, idx, r * 128:(r + 1) * 128],
                                    tri[:],
                                )
                    if weave_fn is not None:
                        weave_fn()
                    for h in heads:
                        if pending[h] is not None:
                            emit_pv(po[h], pending[h][0], pending[h][1],
                                    h, qs, nblocks)
                        pending[h] = (ptc[h], jj)
                for h in heads:
                    emit_pv(po[h], pending[h][0], pending[h][1], h, qs,
                            nblocks)
                    # normalize on device: otile = O^T * bcast(1/sums);
                    # the partition-broadcast runs on GPSIMD (raw sbuf
                    # tensors: the op needs concrete, non-pool APs)
                    i = h % 2
                    nc.vector.reciprocal(rec_raw[i][:],
                                         po[h][HD:HD + 1, :])
                    nc.gpsimd.partition_broadcast(rb_raw[i][:],
                                                  rec_raw[i][:])
                    otile = ot_pool.tile([64, 512], f32, tag="otile",
                                         name=f"otile{h}", bufs=3)
                    nc.vector.tensor_mul(otile[:], po[h][0:HD, :],
                                         rb_raw[i][:])
                    nc.gpsimd.dma_start(
                        outT[h * HD:(h + 1) * HD,
                             qs * 512:(qs + 1) * 512],
                        otile[:],
                    )

            # ---- main emission (self-hosted strips): each strip's
            # projection groups are woven into its OWN attention gaps.
            # Per pair t, only [KT m=t, QT m=t] must precede it; VP groups
            # are woven at 2/chunk during pair 0, ahead of the diagonal
            # PV matmuls that consume them.
            xs = xs0
            for qs in range(NSTRIP):
                qts = qt_pool.tile([128, 4, 512], f32r, tag="qts",
                                   name=f"qts{qs}")
                g = emit_strip_projections(qs, xs, qts)
                # g order: [KT m0..3, VP x4, QT m0..3]
                queue = ([("kq", 0, g[0]), ("kq", 0, g[8])] +
                         [("vp", None, g[4 + i]) for i in range(4)] +
                         [("kq", mm, fn) for mm in (1, 2, 3)
                          for fn in (g[mm], g[8 + mm])])
                # pair-0 requirements upfront
                queue.pop(0)[2]()
                queue.pop(0)[2]()
                if qs + 1 < NSTRIP:
                    xs = load_strip(qs + 1)

                def weave_fn():
                    n = 2 if (queue and queue[0][0] == "vp") else 1
                    for _ in range(n):
                        if queue:
                            queue.pop(0)[2]()

                for t in range(HPC // 2):
                    while queue and any(k == "kq" and mm <= t
                                        for k, mm, _ in queue):
                        queue.pop(0)[2]()
                    attention_pair(2 * t, qs, qts, weave_fn)
                for item in queue:
                    item[2]()

    nc.compile()
    nc.m = get_hw_module(nc.m)
    return nc


_PROGRAM = None


def _program():
    global _PROGRAM
    if _PROGRAM is None:
        _PROGRAM = build_program()
    return _PROGRAM


def make_in_maps(x, Wq, Wk, Wv):
    kk, qq = np.meshgrid(np.arange(128), np.arange(128), indexing="ij")
    tri = (qq >= kk).astype(np.float32)
    in_maps = []
    for c in range(N_CORES):
        b, g = c // 2, c % 2
        sl = slice(g * GD, (g + 1) * GD)
        in_maps.append({
            "xT": np.ascontiguousarray(np.asarray(x)[b].T),
            "wqT": np.ascontiguousarray(np.asarray(Wq).T[:, sl]),
            "wkT": np.ascontiguousarray(np.asarray(Wk).T[:, sl]),
            "wvT": np.ascontiguousarray(np.asarray(Wv).T[:, sl]),
            "tri01": tri,
        })
    return in_maps


def gather(results):
    out = np.empty((B, N, D), np.float32)
    for c in range(N_CORES):
        b, g = c // 2, c % 2
        out[b, :, g * GD:(g + 1) * GD] = results[c]["outT"].T
    return out


def kernel(x, Wq, Wk, Wv):
    nc = _program()
    in_maps = make_in_maps(x, Wq, Wk, Wv)
    res = bass_utils.run_bass_kernel_spmd(nc, in_maps,
                                          core_ids=list(range(N_CORES)))
    return gather(res.results)



# revision 36
# speedup vs baseline: 1.3692x; 1.3692x over previous
"""Multi-head causal attention (b=4, n=2048, d=1024, h=16) on 8 trn2 cores.

Sharding: data-parallel over batch (4) x tensor-parallel over heads (2 groups
of 8 heads).  Core c handles batch c//2, heads 8*(c%2)..8*(c%2)+8.

fp8 redesign (vs the f32r baseline, 262us -> ~191us):
  - All projections are fp8e4m3 DoubleRow matmuls (2 stacked 128-row k-tiles
    per instruction, 0.5 cycles/row).  Host ships x^T and W^T*16 as hi/lo
    e4m3 pairs (lo = quantization residual); strip 0 (queries 0-511, the
    error-sensitive early tokens) uses the 3-term split hi*hi+lo*hi+hi*lo
    (~0.2% err), strips 1-3 use the hi term only (~5% err, which averages
    out over >=512 softmax terms).  W is pre-split into m0-columns + rest
    so the first head's projections start after ~3 small DMAs.
  - Causal masking is a pair of tiny fp8 matmuls (224*I @ -224*tri)
    accumulated into the S PSUM ahead of exp (replaces DVE tri-multiplies).
  - exp (ACT) reads S' = 256*S from PSUM, writes P = 8*exp(S/8) straight to
    fp8 (strips 1-3) or f32r (strip 0).  ACT is the wall: ~154us busy.
  - Strip 0 PV stays f32r in the [65, 512] O^T layout with GPSIMD
    partition-broadcast normalization (exact path, outT0 = O^T).
  - Strips 1-3 PV: fp8 DoubleRow with queries on PSUM partitions:
    out [128q, {V|WS}] accumulates over key-block pairs (P^T stationary, V
    moving, 65-row instructions).  The 4 q-subblock accumulators share one
    PSUM bank; only the very first matmul sets start (bank-granular pending
    zero covers all four).  The ones column carries WS so out = O/sums needs
    no rescale; normalization is a per-partition reciprocal + tensor_scalar
    multiply on DVE, then a per-head [128, 4, 64] DMA (sync/gpsimd queues
    alternating).
  - Schedule: strips 0+1 (and the first two s2 heads) interleave head-wise
    in one phase so strip-0's expensive split projections hide under
    strip-1's ACT-heavy chunks; later strips run head-sequential.  Next-
    strip projection groups are woven 1 matmul at a time into the chunk
    stream via a tagged FIFO with forced drains at dependency points;
    PV/norm work is deferred through a pend FIFO so it never sits between
    a QK and its exp on the in-order PE queue.
  - DMA: dram tensors mirror sbuf tile layouts exactly (one contiguous DMA
    per tile); each HWDGE dma costs a fixed 625ns queue slot and the shared
    DMA_ENGINES device serializes transfers, so loads are few, ordered
    critical-first (wq_m0, wk_m0, x-hi strip 0, x-lo strip 0, wv, W-rest,
    x-hi rest), with outputs mostly on the Pool/SWDGE queue.
  - ~3us of dummy matmuls at t=0 hold the PE p-state ramp so real matmuls
    run at full rate from the start.
"""

import os
import numpy as np
import ml_dtypes

import concourse.bacc as bacc
import concourse.mybir as mybir
import concourse.tile as tile
from concourse import bass_utils
from concourse.bass_interp import get_hw_module

N_CORES = 8
B, N, D = 4, 2048, 1024
HEADS = 16
HPC = 8            # heads per core
HD = 64            # head dim
GD = HPC * HD      # 512 weight columns per core
KC = D // 128      # 8 contraction chunks of in_dim
NB = N // 128      # 16 key blocks
NSTRIP = N // 512  # 4 query strips
HDP = HD + 4       # fp8 V row padded to 4B alignment (65 used + 3 pad)

WS = 16.0                    # weight prescale (W' = W.T * WS)
BEXP = 8.0                   # exp output scale (P = BEXP * exp(S/8))
LN_BEXP = float(np.log(BEXP))
SCALE = 0.125 / (WS * WS)    # exp input scale: S' = WS^2 * S
MC = 224.0                   # mask matmul constant: adds -MC^2 ~ -50k to S'

f32 = mybir.dt.float32
f32r = mybir.dt.float32r
f8 = mybir.dt.float8e4
F8NP = ml_dtypes.float8_e4m3
EXP = mybir.ActivationFunctionType.Exp
DR = mybir.MatmulPerfMode.DoubleRow


def build_program():
    nc = bacc.Bacc("TRN2", target_bir_lowering=False, debug=False,
                   num_devices=N_CORES)
    # dram layouts mirror the sbuf tiles exactly: one contiguous DMA each
    xhd = nc.dram_tensor("xh", [128, KC, N], f8,
                         kind="ExternalInput").ap()
    xl0d = nc.dram_tensor("xl0", [128, KC, 512], f8,
                          kind="ExternalInput").ap()
    # wq/wk split into m0 columns + rest so the first head's projection
    # data arrives in two small contiguous DMAs
    w8q0d = nc.dram_tensor("w8q0", [128, KC, 2, 128], f8,
                           kind="ExternalInput").ap()
    w8qrd = nc.dram_tensor("w8qr", [128, KC, 2, 384], f8,
                           kind="ExternalInput").ap()
    w8k0d = nc.dram_tensor("w8k0", [128, KC, 2, 128], f8,
                           kind="ExternalInput").ap()
    w8krd = nc.dram_tensor("w8kr", [128, KC, 2, 384], f8,
                           kind="ExternalInput").ap()
    w8vd = nc.dram_tensor("w8v", [128, KC, 2, GD], f8,
                          kind="ExternalInput").ap()
    id224d = nc.dram_tensor("id224", [128, 128], f8, kind="ExternalInput").ap()
    trinegd = nc.dram_tensor("trineg", [128, 128], f8,
                             kind="ExternalInput").ap()
    outT0 = nc.dram_tensor("outT0", [GD, 512], f32, kind="ExternalOutput").ap()
    outN = nc.dram_tensor("outN", [N - 512, GD], f32,
                          kind="ExternalOutput").ap()
    # raw (non-pool) sbuf tensors: partition_broadcast needs concrete APs
    rec_raw = [nc.alloc_sbuf_tensor(f"rec_raw{i}", [1, 512], f32).ap()
               for i in range(2)]
    rb_raw = [nc.alloc_sbuf_tensor(f"rb_raw{i}", [64, 512], f32).ap()
              for i in range(2)]
    # per-partition exp bias (= ln BEXP); activation bias must be an AP
    ebias = nc.alloc_sbuf_tensor("ebias", [128, 1], f32).ap()
    nc.gpsimd.memset(ebias, LN_BEXP)
    nc.all_engine_barrier()

    with tile.TileContext(nc) as tc:
        with (
            tc.tile_pool(name="xs", bufs=2) as xs_pool,
            tc.tile_pool(name="w", bufs=1) as w_pool,
            tc.tile_pool(name="big", bufs=1) as big_pool,
            tc.tile_pool(name="qt", bufs=2) as qt_pool,
            tc.tile_pool(name="pt0", bufs=3) as pt0_pool,
            tc.tile_pool(name="pt", bufs=4) as pt_pool,
            tc.tile_pool(name="ot", bufs=2) as ot_pool,
            tc.tile_pool(name="outt", bufs=8) as outt_pool,
            tc.tile_pool(name="rec", bufs=2) as rec_pool,
            tc.tile_pool(name="small", bufs=1) as small_pool,
            tc.tile_pool(name="ps_s", bufs=2, space="PSUM") as ps_s,
            tc.tile_pool(name="ps_proj", bufs=2, space="PSUM") as ps_proj,
            tc.tile_pool(name="ps_pv", bufs=2, space="PSUM") as ps_pv,
        ):
            wq8a = w_pool.tile([128, KC, 2, 128], f8, tag="wq8a")
            wq8b = w_pool.tile([128, KC, 2, 384], f8, tag="wq8b")
            wk8a = w_pool.tile([128, KC, 2, 128], f8, tag="wk8a")
            wk8b = w_pool.tile([128, KC, 2, 384], f8, tag="wk8b")
            wv8 = w_pool.tile([128, KC, 2, GD], f8, tag="wv8")

            def wsel(wa, wb, m, kp, pl):
                if m == 0:
                    return wa[:, 2 * kp:2 * kp + 2, pl, :]
                return wb[:, 2 * kp:2 * kp + 2, pl,
                          (m - 1) * 128:m * 128]
            # interleave wk and strip-0 x chunks: the first KT group consumes
            # them in k order, so it starts after a few DMAs
            # each HWDGE dma costs a fixed 625ns queue slot and the shared
            # DMA_ENGINES device serializes transfers: few DMAs, critical
            # ones (wk, xs0, wq) first
            xh = xs_pool.tile([128, KC, N], f8, tag="xh", name="xh")
            xl0 = xs_pool.tile([128, KC, 512], f8, tag="xl0", name="xl0")
            idt = small_pool.tile([128, 128], f8, tag="idt")
            trt = small_pool.tile([128, 128], f8, tag="trt")
            nc.sync.dma_start(idt[:], id224d[:])
            nc.sync.dma_start(trt[:], trinegd[:])
            # warm the PE p-state during the load phase: ~3us of dummy
            # matmuls starts the ramp clock so real matmuls run at full rate
            pwarm = ps_proj.tile([128, 128], f32, tag="psp", name="pwarm")
            for i in range(30):
                nc.tensor.matmul(pwarm[:], idt[:], trt[:], start=(i == 0),
                                 stop=(i == 29))
            nc.sync.dma_start(wq8a[:], w8q0d[:])
            nc.sync.dma_start(wk8a[:], w8k0d[:])
            nc.sync.dma_start(xh[:, :, 0:512], xhd[:, :, 0:512])
            nc.sync.dma_start(xl0[:], xl0d[:])
            nc.sync.dma_start(wv8[:], w8vd[:])
            nc.sync.dma_start(wk8b[:], w8krd[:])
            nc.sync.dma_start(wq8b[:], w8qrd[:])
            # warm the ACT exp table while input DMAs stream
            warmup = small_pool.tile([1, 1], f32, tag="warmup")
            nc.vector.memset(warmup[:], 0.0)
            nc.scalar.activation(warmup[:], warmup[:], EXP)

            kt = big_pool.tile([128, 4, N], f32r, tag="kt")
            vp0 = big_pool.tile([128, 4, HPC, HD + 1], f32r, tag="vp0")
            vp8 = big_pool.tile([128, NB, HPC, HDP], f8, tag="vp8")
            # ones columns carry WS (only col HD is ever read unwritten)
            nc.gpsimd.memset(vp0[:, :, :, HD].bitcast(f32), WS)
            nc.gpsimd.memset(vp8[:, :, :, HD], WS)

            # x for strips 1-3 (hi plane), loaded up front in halves
            nc.sync.dma_start(xh[:, :, 512:1280], xhd[:, :, 512:1280])
            nc.sync.dma_start(xh[:, :, 1280:2048], xhd[:, :, 1280:2048])

            def emit_group(lhs_fn, rhs_fn, split, copy_fn, name):
                """DoubleRow matmul group accumulating one [128,512] psum
                chunk, returned as single-instruction atoms.
                lhs_fn/rhs_fn(kp, plane) -> stationary/moving APs."""
                ps = ps_proj.tile([128, 512], f32, tag="psp", name=name)
                terms = [(0, 0)] + ([(1, 0), (0, 1)] if split else [])
                mms = [(kp, a, b) for (a, b) in terms for kp in range(4)]
                n_mm = len(mms)
                atoms = []
                for i, (kp, a, b) in enumerate(mms):
                    atoms.append(
                        lambda kp=kp, a=a, b=b, st=(i == 0),
                        sp=(i == n_mm - 1): nc.tensor.matmul(
                            ps[:], lhs_fn(kp, a), rhs_fn(kp, b),
                            start=st, stop=sp, perf_mode=DR))
                atoms.append(lambda: copy_fn(ps))
                return atoms

            def strip_proj_atoms(s, qts):
                """Tagged atoms projecting K^T/Q^T/V for strip s."""
                split = (s == 0)
                base = s * 512

                def xap(kp, pl, c0, c1, base=base):
                    if pl == 1:
                        return xl0[:, 2 * kp:2 * kp + 2, c0:c1]
                    return xh[:, 2 * kp:2 * kp + 2, base + c0:base + c1]
                atoms = []
                for m in range(4):
                    for wab, cp in (
                        ((wk8a, wk8b), lambda ps, m=m, s=s:
                         nc.vector.tensor_copy(
                             kt[:, m, s * 512:(s + 1) * 512], ps[:])),
                        ((wq8a, wq8b), lambda ps, m=m: nc.vector.tensor_copy(
                            qts[:, m, :], ps[:])),
                    ):
                        grp = emit_group(
                            lambda kp, pl, wab=wab, m=m: wsel(
                                wab[0], wab[1], m, kp, pl),
                            lambda kp, pl: xap(kp, pl, 0, 512),
                            split, cp, f"pj{s}m{m}")
                        atoms += [(("kq", s, m), a) for a in grp]
                for i in range(4):
                    j = 4 * s + i

                    def vcopy(ps, j=j):
                        nc.vector.tensor_copy(
                            vp8[:, j, :, 0:HD],
                            ps[:].rearrange("p (h d) -> p h d", h=HPC))
                        if j < 4:
                            nc.vector.tensor_copy(
                                vp0[:, j, :, 0:HD],
                                ps[:].rearrange("p (h d) -> p h d", h=HPC))
                    grp = emit_group(
                        lambda kp, pl, i=i: xap(kp, pl, i * 128,
                                                (i + 1) * 128),
                        lambda kp, pl: wv8[:, 2 * kp:2 * kp + 2, pl, :],
                        split, vcopy, f"pv{s}b{j}")
                    atoms += [(("v", s, j), a) for a in grp]
                return atoms

            queue = []
            pend = []  # deferred PV / PV+norm closures (FIFO)

            def flush(n=1):
                for _ in range(min(n, len(pend))):
                    pend.pop(0)()

            def drain(pred):
                while queue and any(pred(t) for t, _ in queue):
                    queue.pop(0)[1]()

            def weave(n):
                for _ in range(min(n, len(queue))):
                    queue.pop(0)[1]()

            def atoms_by(atoms):
                kq = {m: [a for a in atoms if a[0][0] == "kq"
                          and a[0][2] == m] for m in range(4)}
                v = [a for a in atoms if a[0][0] == "v"]
                return kq, v

            qts = {0: qt_pool.tile([128, 4, 512], f32r, tag="qts",
                                   name="qts0")}
            s0_atoms = strip_proj_atoms(0, qts[0])
            # KT m0 + QT m0 up front, matmuls interleaved so both groups
            # finish as soon as the x chunks land; copies last
            kt0 = [s0_atoms.pop(0) for _ in range(13)]
            qt0 = [s0_atoms.pop(0) for _ in range(13)]
            for a, b in zip(kt0[:12], qt0[:12]):
                a[1]()
                b[1]()
            kt0[12][1]()
            qt0[12][1]()
            qts[1] = qt_pool.tile([128, 4, 512], f32r, tag="qts",
                                  name="qts1")
            k0, v0 = atoms_by(s0_atoms)
            k1, v1 = atoms_by(strip_proj_atoms(1, qts[1]))
            queue.extend(v0 + k1[0] + v1 + k0[1] + k1[1] + k0[2] + k1[2]
                         + k0[3] + k1[3])

            def strip0_head(h, wv_n):
                m, p0 = h // 2, 64 * (h % 2)
                drain(lambda t: t[0] == "kq" and t[1] == 0 and t[2] <= m)
                po = ps_pv.tile([HD + 1, 512], f32, tag="pv", name=f"po{h}")
                # chunk 0: blocks 0 and 1 (block 1 cols 0:128 unread)
                pss = ps_s.tile([128, 2, 512], f32, tag="pss",
                                name=f"s0ps{h}a")
                nc.tensor.matmul(pss[:, 0, :], kt[p0:p0 + 64, m, 0:128],
                                 qts[0][p0:p0 + 64, m, :], start=True,
                                 stop=True)
                nc.tensor.matmul(pss[:, 0, 0:128], idt[:], trt[:],
                                 start=False, stop=False,
                                 skip_group_check=True)
                nc.tensor.matmul(pss[:, 1, :], kt[p0:p0 + 64, m, 128:256],
                                 qts[0][p0:p0 + 64, m, :], start=True,
                                 stop=True)
                nc.tensor.matmul(pss[:, 1, 128:256], idt[:], trt[:],
                                 start=False, stop=False,
                                 skip_group_check=True)
                pta = pt0_pool.tile([128, 2, 512], f32r, tag="pt0",
                                    name=f"pt0{h}a")
                nc.scalar.activation(pta[:], pss[:], EXP, scale=SCALE,
                                     bias=ebias)
                weave(wv_n)
                flush()
                # chunk 1: blocks 2 and 3 (queries 256:512)
                pss2 = ps_s.tile([128, 2, 512], f32, tag="pss",
                                 name=f"s0ps{h}b")
                nc.tensor.matmul(pss2[:, 0, 256:512],
                                 kt[p0:p0 + 64, m, 256:384],
                                 qts[0][p0:p0 + 64, m, 256:512], start=True,
                                 stop=True)
                nc.tensor.matmul(pss2[:, 0, 256:384], idt[:], trt[:],
                                 start=False, stop=False,
                                 skip_group_check=True)
                nc.tensor.matmul(pss2[:, 1, 256:512],
                                 kt[p0:p0 + 64, m, 384:512],
                                 qts[0][p0:p0 + 64, m, 256:512], start=True,
                                 stop=True)
                nc.tensor.matmul(pss2[:, 1, 384:512], idt[:], trt[:],
                                 start=False, stop=False,
                                 skip_group_check=True)
                ptb = pt0_pool.tile([128, 2, 512], f32r, tag="pt0",
                                    name=f"pt0{h}b")
                nc.scalar.activation(ptb[:, :, 256:512], pss2[:, :, 256:512],
                                     EXP, scale=SCALE, bias=ebias)
                weave(wv_n)
                flush()

                def pv_a(h=h, po=po, pta=pta):
                    drain(lambda t: t[0] == "v" and t[1] == 0 and t[2] <= 1)
                    nc.tensor.matmul(po[:, :], vp0[:, 0, h, :], pta[:, 0, :],
                                     start=True, stop=False,
                                     skip_group_check=True)
                    nc.tensor.matmul(po[:, 128:512], vp0[:, 1, h, :],
                                     pta[:, 1, 128:512], start=False,
                                     stop=False, skip_group_check=True)

                def pv_b(h=h, po=po, ptb=ptb):
                    drain(lambda t: t[0] == "v" and t[1] == 0)
                    nc.tensor.matmul(po[:, 256:512], vp0[:, 2, h, :],
                                     ptb[:, 0, 256:512], start=False,
                                     stop=False, skip_group_check=True)
                    nc.tensor.matmul(po[:, 384:512], vp0[:, 3, h, :],
                                     ptb[:, 1, 384:512], start=False,
                                     stop=True, skip_group_check=True)
                    i = h % 2
                    nc.vector.reciprocal(rec_raw[i][:], po[HD:HD + 1, :])
                    nc.gpsimd.partition_broadcast(rb_raw[i][:], rec_raw[i][:])
                    otile = ot_pool.tile([64, 512], f32, tag="otile",
                                         name=f"ot0{h}")
                    nc.vector.tensor_mul(otile[:], po[0:HD, :], rb_raw[i][:])
                    nc.gpsimd.dma_start(
                        outT0[h * HD:(h + 1) * HD, :], otile[:])
                pend.append(pv_a)
                pend.append(pv_b)

            def stripN_head(s, h, wv_n):
                m, p0 = h // 2, 64 * (h % 2)
                drain(lambda t: t[0] == "kq" and t[1] == s and t[2] <= m)
                pv = ps_pv.tile([128, 4, HD + 1], f32, tag="pv",
                                name=f"pv{s}_{h}")
                for c in range(2 * s + 2):
                    last = (c == 2 * s + 1)
                    pss = ps_s.tile([128, 2, 512], f32, tag="pss",
                                    name=f"ps{s}_{h}_{c}")
                    for idx, j in ((0, 2 * c), (1, 2 * c + 1)):
                        r = j - 4 * s
                        qlo = 0 if r < 2 else 256
                        nc.tensor.matmul(
                            pss[:, idx, qlo:512],
                            kt[p0:p0 + 64, m, j * 128:(j + 1) * 128],
                            qts[s][p0:p0 + 64, m, qlo:512],
                            start=True, stop=True)
                        if r >= 0:
                            nc.tensor.matmul(
                                pss[:, idx, r * 128:(r + 1) * 128],
                                idt[:], trt[:], start=False, stop=False,
                                skip_group_check=True)
                    ptc = pt_pool.tile([128, 2, 512], f8, tag="ptc",
                                       name=f"pt{s}_{h}_{c}")
                    if last:
                        nc.scalar.activation(ptc[:, :, 256:512],
                                             pss[:, :, 256:512], EXP,
                                             scale=SCALE, bias=ebias)
                    else:
                        nc.scalar.activation(ptc[:], pss[:], EXP,
                                             scale=SCALE, bias=ebias)
                    weave(wv_n)
                    flush()

                    def pv_chunk(c=c, h=h, pv=pv, ptc=ptc, s=s):
                        drain(lambda t: t[0] == "v" and t[1] <= s
                              and t[2] <= 2 * c + 1)
                        for qb in range(4):
                            nb = 4 * s + qb + 1
                            st = (c == 0 and qb == 0)
                            if 2 * c + 1 < nb:
                                nc.tensor.matmul(
                                    pv[:, qb, :],
                                    ptc[:, :, qb * 128:(qb + 1) * 128],
                                    vp8[:, 2 * c:2 * c + 2, h, 0:HD + 1],
                                    start=st, stop=(c == (nb - 1) // 2),
                                    perf_mode=DR, skip_group_check=True)
                            elif 2 * c == nb - 1:
                                nc.tensor.matmul(
                                    pv[:, qb, :],
                                    ptc[:, 0, qb * 128:(qb + 1) * 128],
                                    vp8[:, 2 * c, h, 0:HD + 1],
                                    start=st, stop=True,
                                    skip_group_check=True)
                    pend.append(pv_chunk)

                def norm(h=h, pv=pv, s=s):
                    rec = rec_pool.tile([128, 4], f32, tag="rec",
                                        name=f"rc{s}_{h}")
                    nc.vector.reciprocal(rec[:], pv[:, :, HD])
                    oht = outt_pool.tile([128, 4, HD], f32, tag="outt",
                                         name=f"ob{s}_{h}")
                    for qb in range(4):
                        nc.vector.tensor_scalar_mul(
                            oht[:, qb, :], pv[:, qb, 0:HD],
                            rec[:, qb:qb + 1])
                    # outN rows (s-1)*512 + qb*128 + p, cols h*64..
                    dst = outN[(s - 1) * 512:s * 512,
                               h * HD:(h + 1) * HD].rearrange(
                        "(qb p) d -> p qb d", p=128)
                    if h % 2 == 1:
                        nc.sync.dma_start(dst, oht[:])
                    else:
                        nc.gpsimd.dma_start(dst, oht[:])
                pend.append(norm)

            # ---- phase A: strips 0 and 1 interleaved, s2 head start ------
            sched = [(0, 0), (0, 1), (1, 0), (0, 2), (1, 1), (0, 3),
                     (1, 2), (0, 4), (1, 3), (0, 5), (1, 4), (0, 6), (1, 5),
                     (0, 7), (2, 0), (1, 6), (2, 1), (1, 7)]
            s2_queued = False
            for i, (s, h) in enumerate(sched):
                if i == 3 and not s2_queued:
                    # queue strip-2 projections mid-phase
                    qts[2] = qt_pool.tile([128, 4, 512], f32r, tag="qts",
                                          name="qts2")
                    queue.extend(strip_proj_atoms(2, qts[2]))
                    s2_queued = True
                if s == 0:
                    strip0_head(h, 5)
                else:
                    stripN_head(s, h, 4)

            # ---- phases B, C: strips 2 and 3 -----------------------------
            nstrip_run = int(os.environ.get("KBISECT_NSTRIP", NSTRIP))
            for s in range(2, nstrip_run):
                if s + 1 < NSTRIP:
                    qts[s + 1] = qt_pool.tile([128, 4, 512], f32r, tag="qts",
                                              name=f"qts{s + 1}")
                    queue.extend(strip_proj_atoms(s + 1, qts[s + 1]))
                for h in range(2 if s == 2 else 0, HPC):
                    stripN_head(s, h, 2)
            flush(len(pend) + 1)
            for _, a in queue:
                a()
            queue.clear()

    nc.compile()
    nc.m = get_hw_module(nc.m)
    return nc


_PROGRAM = None


def _program():
    global _PROGRAM
    if _PROGRAM is None:
        _PROGRAM = build_program()
    return _PROGRAM


def make_in_maps(x, Wq, Wk, Wv):
    kk, qq = np.meshgrid(np.arange(128), np.arange(128), indexing="ij")
    trineg = np.where(kk > qq, -MC, 0.0).astype(np.float32).astype(F8NP)
    id224 = (MC * np.eye(128, dtype=np.float32)).astype(F8NP)

    def split(a):  # [1024, cols] f32 -> [128, KC, 2, cols] hi/lo e4m3
        hi = a.astype(F8NP)
        lo = (a - hi.astype(np.float32)).astype(F8NP)
        s = np.stack([hi, lo], axis=1)  # [1024, 2, cols]
        s = s.reshape(KC, 128, 2, a.shape[1]).transpose(1, 0, 2, 3)
        return np.ascontiguousarray(s)

    in_maps = []
    for c in range(N_CORES):
        b, g = c // 2, c % 2
        sl = slice(g * GD, (g + 1) * GD)
        xT = np.ascontiguousarray(np.asarray(x)[b].T).astype(np.float32)
        xs = split(xT)  # [128, KC, 2, 2048]
        wq = split(np.asarray(Wq).T[:, sl] * WS)
        wk = split(np.asarray(Wk).T[:, sl] * WS)
        in_maps.append({
            "xh": np.ascontiguousarray(xs[:, :, 0, :]),
            "xl0": np.ascontiguousarray(xs[:, :, 1, 0:512]),
            "w8q0": np.ascontiguousarray(wq[:, :, :, 0:128]),
            "w8qr": np.ascontiguousarray(wq[:, :, :, 128:512]),
            "w8k0": np.ascontiguousarray(wk[:, :, :, 0:128]),
            "w8kr": np.ascontiguousarray(wk[:, :, :, 128:512]),
            "w8v": split(np.asarray(Wv).T[:, sl] * WS),
            "id224": id224,
            "trineg": trineg,
        })
    return in_maps


def gather(results):
    out = np.empty((B, N, D), np.float32)
    for c in range(N_CORES):
        b, g = c // 2, c % 2
        sl = slice(g * GD, (g + 1) * GD)
        out[b, 0:512, sl] = results[c]["outT0"].T
        out[b, 512:, sl] = results[c]["outN"]
    return out


def kernel(x, Wq, Wk, Wv):
    nc = _program()
    in_maps = make_in_maps(x, Wq, Wk, Wv)
    res = bass_utils.run_bass_kernel_spmd(nc, in_maps,
                                          core_ids=list(range(N_CORES)))
    return gather(res.results)
